# revision 5
# baseline (speedup 1.0000x reference)
"""Trainium2 Bass kernel for nn_DLUPack (CARAFE-style dynamic upsampling), v2.

Sharding: 8 cores = (batch n in [0,4)) x (output-row-parity s in {0,1});
core (n, s) computes low-res rows hh in [32s, 32s+32) -> all parity-s output rows.

v2 layout: back phase jh-packed on 128 partitions, p = 64*jh + w.
  ref[n, c, 2y+i, 2x+j]: for core (n,s), y = h0 + 16*jh + m (h0=32s),
  out DRAM row r' = 4m + w//16, dcol = 8*(w%16) + 2u + jh, host: out[n,:,s::2].

Pipeline per core:
  1. compressor 1x1 conv (PE) -> cx [64, 38, 66] fp16
  2. offset+mask 3x3 convs (9 accumulated MMs x 6 groups) -> psum [57, 384]
  3. 16 po transposes -> deltT128 [128, 16, 8]; W9 indicator chain (DVE)
  4. 20 exp transposes -> expT128 [128, 20, 25] f32; softmax; msm4 [128,20,25,4] fp16
  5. +-1 w-shift variants of msm4 via SBUF-SBUF DMA (within 64-halves)
  6. kernc [128, 16m, 25k, 4u] assembly: 17 TT ops x 2 blocks (DVE, fp16 2x)
  7. kbf partition-shift variants (4 DMAs/blk); prep -> data_all [128, 16, 100]
  8. per m: local_scatter [128, 1280] (GPSIMD); per (jh, ch): 5 accumulated MMs
     lhsT=xT2[64jh.., 128c] rhs=banded[64jh.., ki*256..] -> psum [128c, 256px]
  9. ACT evac (fp16, jh-interleaved cols) -> rb group tile; 1 out DMA per (4m, ch)
"""
import sys
import numpy as np

sys.path.insert(0, '/opt/trn_rl_repo')

import ml_dtypes  # noqa: E402,F401
from contextlib import ExitStack  # noqa: E402

import concourse.bass as bass  # noqa: E402
import concourse.tile as tile  # noqa: E402
from concourse import mybir, bacc  # noqa: E402
from concourse.bass_utils import run_bass_kernel_spmd  # noqa: E402

F32 = mybir.dt.float32
FP16 = mybir.dt.float16
I16 = mybir.dt.int16
AF = mybir.ActivationFunctionType
OP = mybir.AluOpType

N, C, H, W = 4, 256, 64, 64
NWARM = 24


def _ap(base, off_elems, dims):
    return bass.AP(tensor=base.tensor, offset=base.offset + off_elems, ap=[list(d) for d in dims])


def build_scatter_table():
    # banded[p=64jh+pp, ki*256 + 4*w + u] = kernc[64jh + w, m, ki*5+(4-b), u],
    # w = pp + b - 2; data_all[p, (b*5+ki)*4+u] laid out by prep.
    idx = -np.ones((128, 100), np.int16)
    for p in range(128):
        pp = p % 64
        for b in range(5):
            w = pp + b - 2
            if not (0 <= w < 64):
                continue
            for ki in range(5):
                for u in range(4):
                    idx[p, (b * 5 + ki) * 4 + u] = ki * 256 + 4 * w + u
    return idx


# params [128, 40] f32 column map
P_WVEC, P_W63, P_HROW, P_Y63, P_BCOMP, P_BCO, P_E3 = 0, 1, 2, 18, 34, 35, 36


def build_program():
    nc = bacc.Bacc(None, target_bir_lowering=False, debug=True)

    xwin = nc.declare_dram_parameter('xwin', [2, 128, 38 * 64], FP16, isOutput=False)
    xT2 = nc.declare_dram_parameter('xT2', [128, 20 * 256], FP16, isOutput=False)
    wc = nc.declare_dram_parameter('wc', [128, 2 * 64], FP16, isOutput=False)
    wk = nc.declare_dram_parameter('wk', [128, 6 * 57], FP16, isOutput=False)
    params = nc.declare_dram_parameter('params', [128, 40], F32, isOutput=False)
    ident = nc.declare_dram_parameter('ident', [128, 128], F32, isOutput=False)
    idxt = nc.declare_dram_parameter('idxt', [128, 100], I16, isOutput=False)
    zed = nc.declare_dram_parameter('zed', [2, 3600], FP16, isOutput=False)
    outp = nc.declare_dram_parameter('outp', [256, 64 * 128], FP16, isOutput=True)

    with tile.TileContext(nc) as tc, ExitStack() as ctx:
        sing = ctx.enter_context(tc.tile_pool(name='sing', bufs=1))
        work = ctx.enter_context(tc.tile_pool(name='work', bufs=1))
        band = ctx.enter_context(tc.tile_pool(name='band', bufs=4))
        rbp = ctx.enter_context(tc.tile_pool(name='rbp', bufs=2))
        psum = ctx.enter_context(tc.psum_pool(name='ps', bufs=2))
        psc = ctx.enter_context(tc.psum_pool(name='psc', bufs=6))

        def load(shape, dtype, src, name, eng=None):
            t = sing.tile(shape, dtype, name=name)
            (eng or nc.sync).dma_start(out=t[:], in_=src[:])
            return t

        id_sb = load([128, 128], F32, ident, 'id')
        # xwin split into 4 DMAs across two issue queues for transfer parallelism
        xwin_sb = sing.tile([128, 2, 38 * 64], FP16)
        for cg_ in range(2):
            for rh in range(2):
                eng = nc.sync if rh == 0 else nc.scalar
                r0, r1 = (0, 1216) if rh == 0 else (1216, 2432)
                eng.dma_start(out=_ap(xwin_sb[:], cg_ * 2432 + r0, [[2 * 2432, 128], [1, r1 - r0]]),
                              in_=_ap(xwin[:], cg_ * 128 * 2432 + r0, [[2432, 128], [1, r1 - r0]]))
        wc_sb = load([128, 2, 64], FP16, wc, 'wc')
        xT2_sb = sing.tile([128, 20 * 256], FP16)
        for rh in range(2):
            eng = nc.scalar if rh == 0 else nc.sync
            r0, r1 = (0, 2560) if rh == 0 else (2560, 5120)
            eng.dma_start(out=_ap(xT2_sb[:], r0, [[5120, 128], [1, r1 - r0]]),
                          in_=_ap(xT2[:], r0, [[5120, 128], [1, r1 - r0]]))
        wk_sb = load([128, 6 * 57], FP16, wk, 'wk', nc.scalar)
        par_sb = load([128, 40], F32, params, 'par')
        idx_sb = load([128, 100], I16, idxt, 'idx', nc.scalar)

        # PE warm-up while input DMAs land; dummy ACT pulls the table load early
        pw = psum.tile([128, 512], F32, name='warm', tag='front')
        dumt = work.tile([1, 4], F32, name='dumt')
        nc.scalar.activation(out=dumt[:], in_=id_sb[0:1, 0:4], func=AF.Copy, scale=1.0)
        for _ in range(NWARM):
            nc.tensor.matmul(pw[0:64, 0:64], id_sb[:, 0:64], id_sb[:, 0:64], start=True, stop=True)

        wvec = par_sb[:, P_WVEC:P_WVEC + 1]
        w63 = par_sb[:, P_W63:P_W63 + 1]
        bcomp = par_sb[0:64, P_BCOMP:P_BCOMP + 1]
        bker = _ap(par_sb[:], 32 * 40 + P_BCO, [[40, 25], [1, 1]])
        boff = par_sb[0:8, P_BCO:P_BCO + 1]
        hrow_bc = _ap(par_sb[:], P_HROW, [[40, 128], [1, 16], [0, 4]])
        y63_bc = _ap(par_sb[:], P_Y63, [[40, 128], [1, 16], [0, 4]])

        # hoisted shifted-variant buffers; edge partitions zeroed once (gpsimd)
        msm4 = work.tile([128, 20, 25, 4], FP16)
        msm4_p1 = work.tile([128, 20, 25, 4], FP16)   # [p] = msm4[p+1] within half
        msm4_m1 = work.tile([128, 20, 25, 4], FP16)   # [p] = msm4[p-1] within half
        for jh in range(2):
            nc.gpsimd.dma_start(
                out=_ap(msm4_p1[:], (jh * 64 + 63) * 2000, [[2000, 1], [1, 2000]]),
                in_=_ap(zed[:], 0, [[3600, 1], [1, 2000]]))
            nc.gpsimd.dma_start(
                out=_ap(msm4_m1[:], jh * 64 * 2000, [[2000, 1], [1, 2000]]),
                in_=_ap(zed[:], 0, [[3600, 1], [1, 2000]]))
        kernc = work.tile([128, 16 * 100], FP16)
        kbf = {0: kernc}
        for d in (-2, -1, 1, 2):
            kbf[d] = work.tile([128, 16 * 100], FP16, name=f'kbf{d}')
            for jh in range(2):
                if d > 0:
                    nc.gpsimd.dma_start(
                        out=_ap(kbf[d][:], (jh * 64 + 64 - d) * 1600, [[1600, d], [1, 1600]]),
                        in_=_ap(zed[:], 0, [[3600, d], [1, 1600]]))
                else:
                    nc.gpsimd.dma_start(
                        out=_ap(kbf[d][:], jh * 64 * 1600, [[1600, -d], [1, 1600]]),
                        in_=_ap(zed[:], 0, [[3600, -d], [1, 1600]]))

        # ---- 1. compressor ----
        # cx_sb [128, 38, 66]: lower half = cx rows; upper half = cx shifted
        # down one h-row (slot h holds row h+1) so taps (dy=0, dy=1) pack
        # into one 128-deep contraction.
        cx_sb = work.tile([128, 38, 66], FP16)
        nc.vector.memset(_ap(cx_sb[:], 0, [[38 * 66, 128], [66, 38], [1, 1]]), 0.0)
        nc.vector.memset(_ap(cx_sb[:], 65, [[38 * 66, 128], [66, 38], [1, 1]]), 0.0)
        for grp in range(5):
            g0 = grp * 8
            rows = min(8, 38 - g0)
            nn = rows * 64
            pcs = psum.tile([64, 512], F32, name=f'cmp{grp}', tag='front')
            for cg in range(2):
                nc.tensor.matmul(pcs[:, :nn], wc_sb[:, cg, :],
                                 xwin_sb[:, cg, g0 * 64:g0 * 64 + nn],
                                 start=(cg == 0), stop=(cg == 1))
            nc.scalar.activation(
                out=_ap(cx_sb[:], g0 * 66 + 1, [[38 * 66, 64], [66, rows], [1, 64]]),
                in_=_ap(pcs[:], 0, [[512, 64], [64, rows], [1, 64]]),
                func=AF.Identity, bias=bcomp, scale=1.0)
            r0 = max(g0, 1)
            cnt = (g0 + rows - r0) * 66
            nc.gpsimd.dma_start(
                out=_ap(cx_sb[:], 64 * 2508 + (r0 - 1) * 66, [[2508, 64], [1, cnt]]),
                in_=_ap(cx_sb[:], r0 * 66, [[2508, 64], [1, cnt]]))

        # ---- 2. offset+mask convs: 6 MMs (3 tap-pairs + 3 singles) ----
        # expS [25, t20, jh2, 64]: slot (t, jh) = conv row h = t + 16*jh
        # (h in [16,20) stored twice). offS [8, h'16, jh2, 64]: y = h' + 16*jh.
        expS = work.tile([25, 20, 2, 64], F32)
        offS = work.tile([8, 16, 2, 64], F32)
        for grp in range(6):
            g0 = grp * 6
            nn = 6 * 64
            pcs = psum.tile([57, 384], F32, name=f'off{grp}', tag='front')
            for s in range(6):
                if s < 3:  # pair: lower tap (0,s), upper tap (1,s)
                    lhsT = _ap(wk_sb[:], s * 57, [[6 * 57, 128], [1, 57]])
                    rhs = _ap(cx_sb[:], g0 * 66 + s, [[38 * 66, 128], [66, 6], [1, 64]])
                else:      # single: tap (2, s-3), lower half only
                    lhsT = _ap(wk_sb[:], s * 57, [[6 * 57, 64], [1, 57]])
                    rhs = _ap(cx_sb[:], (g0 + 2) * 66 + (s - 3),
                              [[38 * 66, 64], [66, 6], [1, 64]])
                nc.tensor.matmul(pcs[:, :nn], lhsT, rhs,
                                 start=(s == 0), stop=(s == 5))
            for jh in range(2):
                h_lo = max(g0, 20 * jh - 4)      # jh0: t=h in [0,20); jh1: t=h-16
                h_hi = min(g0 + 6, 20 + 16 * jh)
                if h_lo < h_hi:
                    nc.scalar.activation(
                        out=_ap(expS[:], (h_lo - 16 * jh) * 128 + jh * 64,
                                [[2560, 25], [128, h_hi - h_lo], [1, 64]]),
                        in_=_ap(pcs[:], 32 * 384 + (h_lo - g0) * 64,
                                [[384, 25], [64, h_hi - h_lo], [1, 64]]),
                        func=AF.Exp, bias=bker, scale=1.0)
                y_lo = max(g0 - 2, 16 * jh)
                y_hi = min(g0 + 4, 16 + 16 * jh)
                if y_lo < y_hi:
                    nc.vector.tensor_scalar(
                        out=_ap(offS[:], (y_lo - 16 * jh) * 128 + jh * 64,
                                [[2048, 8], [128, y_hi - y_lo], [1, 64]]),
                        in0=_ap(pcs[:], (y_lo + 2 - g0) * 64,
                                [[384, 8], [64, y_hi - y_lo], [1, 64]]),
                        scalar1=boff, scalar2=None, op0=OP.add)

        # ---- 3. offset transposes -> deltT128 [128, 16 h', 8 ch] ----
        po = psum.tile([128, 512], F32, name='po', tag='front')
        for hp in range(16):
            nc.tensor.transpose(po[:, hp * 8:hp * 8 + 8],
                                _ap(offS[:], hp * 128, [[2048, 8], [1, 128]]),
                                id_sb[0:8, 0:8])
        deltT = work.tile([128, 16, 8], FP16)
        nc.scalar.activation(out=deltT[:], in_=_ap(po[:], 0, [[512, 128], [1, 128]]),
                             func=AF.Copy, scale=1.0)

        # ---- 4. W9 indicator chain on [128, 64] ----
        def dview(chbase):
            return _ap(deltT[:], chbase, [[128, 128], [8, 16], [1, 4]])

        def wt(nm):
            return work.tile([128, 64], FP16, name=nm)

        t1, t2 = wt('t1'), wt('t2')
        gxc, x0r, wxt, omwx, x1r = wt('gxc'), wt('x0r'), wt('wxt'), wt('omwx'), wt('x1r')
        gyc, y0r, wyt, omwy, y1r = wt('gyc'), wt('y0r'), wt('wyt'), wt('omwy'), wt('y1r')
        ia, ib = wt('ia'), wt('ib')
        cwx = work.tile([128, 3, 64], FP16)
        rwy = work.tile([128, 3, 64], FP16)
        W9b = work.tile([128, 9, 64], FP16)

        def r4(ap):
            return _ap(ap, 0, [[64, 128], [4, 16], [1, 4]])

        nc.vector.tensor_scalar(out=t1[:], in0=dview(0), scalar1=wvec, scalar2=None, op0=OP.add)
        nc.vector.tensor_scalar(out=t2[:], in0=t1[:], scalar1=0.0, scalar2=63.0, op0=OP.max, op1=OP.min)
        nc.vector.tensor_scalar(out=gxc[:], in0=t2[:], scalar1=wvec, scalar2=None, op0=OP.subtract)
        nc.vector.tensor_scalar(out=x0r[:], in0=gxc[:], scalar1=0.0, scalar2=-1.0, op0=OP.is_lt, op1=OP.mult)
        nc.vector.tensor_tensor(out=wxt[:], in0=gxc[:], in1=x0r[:], op=OP.subtract)
        nc.vector.tensor_scalar(out=omwx[:], in0=wxt[:], scalar1=-1.0, scalar2=1.0, op0=OP.mult, op1=OP.add)
        nc.vector.tensor_scalar(out=x1r[:], in0=x0r[:], scalar1=1.0, scalar2=w63, op0=OP.add, op1=OP.min)

        nc.vector.tensor_tensor(out=r4(t1[:]), in0=dview(4), in1=hrow_bc, op=OP.add)
        nc.vector.tensor_scalar(out=t2[:], in0=t1[:], scalar1=0.0, scalar2=63.0, op0=OP.max, op1=OP.min)
        nc.vector.tensor_tensor(out=r4(gyc[:]), in0=r4(t2[:]), in1=hrow_bc, op=OP.subtract)
        nc.vector.tensor_scalar(out=y0r[:], in0=gyc[:], scalar1=0.0, scalar2=-1.0, op0=OP.is_lt, op1=OP.mult)
        nc.vector.tensor_tensor(out=wyt[:], in0=gyc[:], in1=y0r[:], op=OP.subtract)
        nc.vector.tensor_scalar(out=omwy[:], in0=wyt[:], scalar1=-1.0, scalar2=1.0, op0=OP.mult, op1=OP.add)
        nc.vector.tensor_scalar(out=t1[:], in0=y0r[:], scalar1=1.0, scalar2=None, op0=OP.add)
        nc.vector.tensor_tensor(out=r4(y1r[:]), in0=r4(t1[:]), in1=y63_bc, op=OP.min)

        # batched indicators: all 3 tap offsets at once on [128, 3, 64]
        e3_bc = _ap(par_sb[:], P_E3, [[40, 128], [1, 3], [0, 64]])
        ia3 = work.tile([128, 3, 64], FP16, name='ia3')
        ib3 = work.tile([128, 3, 64], FP16, name='ib3')

        def bc3(t):
            return _ap(t[:], 0, [[64, 128], [0, 3], [1, 64]])

        for r0, r1, w0, w1, outt in ((x0r, x1r, omwx, wxt, cwx), (y0r, y1r, omwy, wyt, rwy)):
            nc.vector.tensor_tensor(out=ia3[:], in0=bc3(r0), in1=e3_bc, op=OP.is_equal)
            nc.vector.tensor_tensor(out=ib3[:], in0=bc3(r1), in1=e3_bc, op=OP.is_equal)
            nc.vector.tensor_tensor(out=ia3[:], in0=ia3[:], in1=bc3(w0), op=OP.mult)
            nc.vector.tensor_tensor(out=ib3[:], in0=ib3[:], in1=bc3(w1), op=OP.mult)
            nc.vector.tensor_tensor(out=outt[:], in0=ia3[:], in1=ib3[:], op=OP.add)
        for iy in range(3):
            for ix in range(3):
                nc.vector.tensor_tensor(
                    out=_ap(W9b[:], (iy * 3 + ix) * 64, [[9 * 64, 128], [1, 64]]),
                    in0=rwy[:, iy, :], in1=cwx[:, ix, :], op=OP.mult)

        # ---- 5. exp transposes -> expT128 [128, 20 t, 25 k]; softmax ----
        pt = psum.tile([128, 512], F32, name='pt', tag='front')
        for t in range(20):
            nc.tensor.transpose(pt[:, t * 25:t * 25 + 25],
                                _ap(expS[:], t * 128, [[2560, 25], [1, 128]]),
                                id_sb[0:25, 0:25])
        expT = work.tile([128, 20, 25], F32)
        nc.scalar.activation(out=expT[:], in_=_ap(pt[:], 0, [[512, 128], [1, 500]]),
                             func=AF.Copy, scale=1.0)
        sumT = work.tile([128, 20], F32)
        nc.vector.tensor_reduce(out=sumT[:], in_=expT[:], axis=mybir.AxisListType.X, op=OP.add)
        recT = work.tile([128, 20], F32)
        nc.vector.reciprocal(out=recT[:], in_=sumT[:])
        nc.vector.tensor_tensor(
            out=msm4[:],
            in0=_ap(expT[:], 0, [[500, 128], [25, 20], [1, 25], [0, 4]]),
            in1=_ap(recT[:], 0, [[20, 128], [1, 20], [0, 25], [0, 4]]), op=OP.mult)
        shift_engs = (nc.gpsimd, nc.sync, nc.scalar, nc.gpsimd)
        for jh in range(2):
            b0 = jh * 64 * 2000
            shift_engs[2 * jh].dma_start(
                out=_ap(msm4_p1[:], b0, [[2000, 63], [1, 2000]]),
                in_=_ap(msm4[:], b0 + 2000, [[2000, 63], [1, 2000]]))
            shift_engs[2 * jh + 1].dma_start(
                out=_ap(msm4_m1[:], b0 + 2000, [[2000, 63], [1, 2000]]),
                in_=_ap(msm4[:], b0, [[2000, 63], [1, 2000]]))

        # ---- 6-9. kernc assembly + banded + carafe, 4 blocks of 4 m ----
        msm_by_ex = {-1: msm4_m1, 0: msm4, 1: msm4_p1}
        data_all = work.tile([128, 16, 100], FP16)
        pbuf = [work.tile([128, 400], FP16, name=f'pb{t}') for t in range(9)]

        def emit_asm(m0, gm):
            # 9 independent products, then a pairwise reduction tree
            kv = _ap(kernc[:], m0 * 100, [[1600, 128], [100, gm], [4, 25], [1, 4]])

            def pv(t):
                return _ap(pbuf[t][:], 0, [[400, 128], [100, gm], [4, 25], [1, 4]])

            for t, (ey, ex) in enumerate((ey, ex) for ey in (-1, 0, 1) for ex in (-1, 0, 1)):
                mv = _ap(msm_by_ex[ex][:], (2 + ey + m0) * 100,
                         [[2000, 128], [100, gm], [4, 25], [1, 4]])
                wv = _ap(W9b[:], t * 64 + m0 * 4,
                         [[9 * 64, 128], [4, gm], [0, 25], [1, 4]])
                nc.vector.tensor_tensor(out=pv(t), in0=wv, in1=mv, op=OP.mult)
            for a, b in ((0, 1), (2, 3), (4, 5), (6, 7), (0, 2), (4, 6), (0, 4)):
                nc.vector.tensor_tensor(out=pv(a), in0=pv(a), in1=pv(b), op=OP.add)
            nc.vector.tensor_tensor(out=kv, in0=pv(0), in1=pv(8), op=OP.add)

        def emit_kbf(m0, gm):
            engs = (nc.sync, nc.scalar, nc.gpsimd, nc.gpsimd)
            for i, d in enumerate((-2, -1, 1, 2)):
                eng = engs[i]
                for jh in range(2):
                    b0 = jh * 64 * 1600 + m0 * 100
                    if d > 0:
                        eng.dma_start(
                            out=_ap(kbf[d][:], b0, [[1600, 64 - d], [1, gm * 100]]),
                            in_=_ap(kernc[:], b0 + d * 1600, [[1600, 64 - d], [1, gm * 100]]))
                    else:
                        eng.dma_start(
                            out=_ap(kbf[d][:], b0 - d * 1600, [[1600, 64 + d], [1, gm * 100]]),
                            in_=_ap(kernc[:], b0, [[1600, 64 + d], [1, gm * 100]]))

        def emit_prep(m0, gm):
            for b in range(5):
                nc.vector.tensor_copy(
                    out=_ap(data_all[:], m0 * 100 + b * 20,
                            [[1600, 128], [100, gm], [4, 5], [1, 4]]),
                    in_=_ap(kbf[b - 2][:], m0 * 100 + (4 - b) * 4,
                            [[1600, 128], [100, gm], [20, 5], [1, 4]]))

        rb_t = [None, None]

        def emit_m(m):
            banded = band.tile([128, 1280], FP16, name=f'band_{m}', tag='band')
            nc.gpsimd.local_scatter(out_ap=banded[:], data_ap=data_all[:, m, :],
                                    idxs_ap=idx_sb[:], channels=128, num_elems=1280,
                                    num_idxs=100)
            if m % 2 == 0:
                g = m // 2
                for ch in range(2):
                    rb_t[ch] = rbp.tile([128, 2 * 512], FP16, name=f'rb_{g}_{ch}', tag=f'rb{ch}')
            for jh in range(2):
                for ch in range(2):
                    pcs = psc.tile([128, 256], F32, name=f'pcs_{m}_{jh}_{ch}', tag='pcs')
                    for ki in range(5):
                        lhsT = _ap(xT2_sb[:], jh * 64 * 5120 + (m + ki) * 256 + ch * 128,
                                   [[5120, 64], [1, 128]])
                        rhs = _ap(banded[:], jh * 64 * 1280 + ki * 256, [[1280, 64], [1, 256]])
                        nc.tensor.matmul(pcs[:], lhsT, rhs, start=(ki == 0), stop=(ki == 4))
                    out_ap = _ap(rb_t[ch][:], (m % 2) * 512 + jh,
                                 [[2 * 512, 128], [128, 4], [8, 16], [2, 4]])
                    in_ap = _ap(pcs[:], 0, [[256, 128], [64, 4], [4, 16], [1, 4]])
                    if jh == 1 and m >= 8:
                        nc.vector.tensor_copy(out=out_ap, in_=in_ap)
                    else:
                        nc.scalar.activation(out=out_ap, in_=in_ap, func=AF.Copy, scale=1.0)
            if m % 2 == 1:
                for ch in range(2):
                    nc.sync.dma_start(
                        out=_ap(outp[:], ch * 128 * 8192 + 4 * (m - 1) * 128,
                                [[8192, 128], [128, 8], [1, 128]]),
                        in_=rb_t[ch][:])

        for m0, gm in ((0, 2), (2, 2), (4, 4), (8, 4), (12, 4)):
            emit_asm(m0, gm)
            emit_kbf(m0, gm)
            emit_prep(m0, gm)
            for m in range(m0, m0 + gm):
                emit_m(m)
    nc.finalize()
    return nc


_PROGRAM = None
_SCAT = build_scatter_table()


def _get_program():
    global _PROGRAM
    if _PROGRAM is None:
        _PROGRAM = build_program()
    return _PROGRAM


def _prep_core_inputs(inputs, n, s):
    bf = np.float16
    x = np.asarray(inputs['x'][n], np.float32)
    h0 = 32 * s
    xw = np.zeros((C, 38, W), np.float32)
    for i, g in enumerate(range(h0 - 3, h0 + 35)):
        if 0 <= g < H:
            xw[:, i] = x[:, g]
    xwin = np.ascontiguousarray(xw.reshape(2, 128, 38 * 64)).astype(bf)
    xT2 = np.zeros((128, 20, C), np.float32)
    for jh in range(2):
        base = h0 + 16 * jh - 2
        for i in range(20):
            g = base + i
            if 0 <= g < H:
                xT2[64 * jh:64 * jh + 64, i] = x[:, g].T
    xT2 = np.ascontiguousarray(xT2.reshape(128, 20 * 256)).astype(bf)
    w_comp = np.asarray(inputs['w_comp'], np.float32)[:, :, 0, 0]
    wc = np.zeros((2, 128, 64), np.float32)
    for cg in range(2):
        wc[cg] = w_comp[:, cg * 128:(cg + 1) * 128].T
    wc = np.ascontiguousarray(wc.transpose(1, 0, 2).reshape(128, 2 * 64)).astype(bf)
    w_ker = np.asarray(inputs['w_ker'], np.float32)
    w_off = np.asarray(inputs['w_off'], np.float32)
    wkT = np.zeros((9, 64, 57), np.float32)
    for t in range(9):
        wkT[t, :, 0:8] = w_off[:, :, t // 3, t % 3].T
        wkT[t, :, 32:57] = w_ker[:, :, t // 3, t % 3].T
    wk = np.zeros((128, 6, 57), np.float32)
    for s, t in enumerate((0, 1, 2, 6, 7, 8)):
        wk[0:64, s] = wkT[t]
    for s, t in enumerate((3, 4, 5)):
        wk[64:128, s] = wkT[t]
    wk = np.ascontiguousarray(wk.reshape(128, 6 * 57)).astype(bf)

    par = np.zeros((128, 40), np.float32)
    p = np.arange(128)
    wv = (p % 64).astype(np.float32)
    jh = (p // 64).astype(np.float32)
    par[:, P_WVEC] = wv
    par[:, P_W63] = 63.0 - wv
    hh = h0 + 16.0 * jh[:, None] + np.arange(16, dtype=np.float32)[None, :]
    par[:, P_HROW:P_HROW + 16] = hh
    par[:, P_Y63:P_Y63 + 16] = 63.0 - hh
    par[0:64, P_BCOMP] = np.asarray(inputs['b_comp'], np.float32)
    bcov = np.zeros(128, np.float32)
    bcov[0:8] = np.asarray(inputs['b_off'], np.float32)
    bcov[32:57] = np.asarray(inputs['b_ker'], np.float32)
    par[:, P_BCO] = bcov
    par[:, P_E3:P_E3 + 3] = np.array([-1.0, 0.0, 1.0], np.float32)

    return {
        'xwin': xwin, 'xT2': xT2, 'wc': wc, 'wk': wk, 'params': par,
        'ident': np.eye(128, dtype=np.float32),
        'idxt': _SCAT,
        'zed': np.zeros((2, 3600), np.float16),
    }


def kernel(**inputs):
    nc = _get_program()
    core_ids = list(range(8))
    in_maps = [_prep_core_inputs(inputs, cid // 2, cid % 2) for cid in core_ids]
    res = run_bass_kernel_spmd(nc, in_maps, core_ids)
    out = np.zeros((N, C, 128, 128), np.float32)
    for cid in core_ids:
        n, s = cid // 2, cid % 2
        op = np.asarray(res.results[cid]['outp']).astype(np.float32).reshape(256, 64, 128)
        out[n, :, s::2] = op
    return out


if __name__ == '__main__':
    d = np.load('/root/problem/ref_io.npz')
    inp = {k: d[k] for k in ('x', 'w_comp', 'b_comp', 'w_ker', 'b_ker', 'w_off', 'b_off')}
    out = kernel(**inp)
    ref = d['out']
    err = np.abs(out - ref).max()
    print('max abs err:', err, 'rel:', err / np.abs(ref).max())


# revision 7
# speedup vs baseline: 1.0196x; 1.0196x over previous
"""Trainium2 Bass kernel for nn_DLUPack (CARAFE-style dynamic upsampling), v2.

Sharding: 8 cores = (batch n in [0,4)) x (output-row-parity s in {0,1});
core (n, s) computes low-res rows hh in [32s, 32s+32) -> all parity-s output rows.

v2 layout: back phase jh-packed on 128 partitions, p = 64*jh + w.
  ref[n, c, 2y+i, 2x+j]: for core (n,s), y = h0 + 16*jh + m (h0=32s),
  out DRAM row r' = 4m + w//16, dcol = 8*(w%16) + 2u + jh, host: out[n,:,s::2].

Pipeline per core:
  1. compressor 1x1 conv (PE) -> cx [64, 38, 66] fp16
  2. offset+mask 3x3 convs (9 accumulated MMs x 6 groups) -> psum [57, 384]
  3. 16 po transposes -> deltT128 [128, 16, 8]; W9 indicator chain (DVE)
  4. 20 exp transposes -> expT128 [128, 20, 25] f32; softmax; msm4 [128,20,25,4] fp16
  5. +-1 w-shift variants of msm4 via SBUF-SBUF DMA (within 64-halves)
  6. kernc [128, 16m, 25k, 4u] assembly: 17 TT ops x 2 blocks (DVE, fp16 2x)
  7. kbf partition-shift variants (4 DMAs/blk); prep -> data_all [128, 16, 100]
  8. per m: local_scatter [128, 1280] (GPSIMD); per (jh, ch): 5 accumulated MMs
     lhsT=xT2[64jh.., 128c] rhs=banded[64jh.., ki*256..] -> psum [128c, 256px]
  9. ACT evac (fp16, jh-interleaved cols) -> rb group tile; 1 out DMA per (4m, ch)
"""
import sys
import numpy as np

sys.path.insert(0, '/opt/trn_rl_repo')

import ml_dtypes  # noqa: E402,F401
from contextlib import ExitStack  # noqa: E402

import concourse.bass as bass  # noqa: E402
import concourse.tile as tile  # noqa: E402
from concourse import mybir, bacc  # noqa: E402
from concourse.bass_utils import run_bass_kernel_spmd  # noqa: E402

F32 = mybir.dt.float32
FP16 = mybir.dt.float16
I16 = mybir.dt.int16
AF = mybir.ActivationFunctionType
OP = mybir.AluOpType

N, C, H, W = 4, 256, 64, 64
NWARM = 24


def _ap(base, off_elems, dims):
    return bass.AP(tensor=base.tensor, offset=base.offset + off_elems, ap=[list(d) for d in dims])


def build_scatter_table():
    # banded[p=64jh+pp, ki*256 + 4*w + u] = kernc[64jh + w, m, ki*5+(4-b), u],
    # w = pp + b - 2; data_all[p, (b*5+ki)*4+u] laid out by prep.
    idx = -np.ones((128, 100), np.int16)
    for p in range(128):
        pp = p % 64
        for b in range(5):
            w = pp + b - 2
            if not (0 <= w < 64):
                continue
            for ki in range(5):
                for u in range(4):
                    idx[p, (b * 5 + ki) * 4 + u] = ki * 256 + 4 * w + u
    return idx


# params [128, 40] f32 column map
P_WVEC, P_W63, P_HROW, P_Y63, P_BCOMP, P_BCO, P_E3 = 0, 1, 2, 18, 34, 35, 36


def build_program():
    nc = bacc.Bacc(None, target_bir_lowering=False, debug=True)

    xwin = nc.declare_dram_parameter('xwin', [2, 128, 38 * 64], FP16, isOutput=False)
    xT2 = nc.declare_dram_parameter('xT2', [128, 20 * 256], FP16, isOutput=False)
    wc = nc.declare_dram_parameter('wc', [128, 2 * 64], FP16, isOutput=False)
    wk = nc.declare_dram_parameter('wk', [128, 6 * 57], FP16, isOutput=False)
    params = nc.declare_dram_parameter('params', [128, 40], F32, isOutput=False)
    ident = nc.declare_dram_parameter('ident', [128, 128], F32, isOutput=False)
    idxt = nc.declare_dram_parameter('idxt', [128, 100], I16, isOutput=False)
    zed = nc.declare_dram_parameter('zed', [2, 3600], FP16, isOutput=False)
    outp = nc.declare_dram_parameter('outp', [256, 64 * 128], FP16, isOutput=True)

    with tile.TileContext(nc) as tc, ExitStack() as ctx:
        sing = ctx.enter_context(tc.tile_pool(name='sing', bufs=1))
        work = ctx.enter_context(tc.tile_pool(name='work', bufs=1))
        band = ctx.enter_context(tc.tile_pool(name='band', bufs=4))
        rbp = ctx.enter_context(tc.tile_pool(name='rbp', bufs=2))
        psum = ctx.enter_context(tc.psum_pool(name='ps', bufs=2))
        psc = ctx.enter_context(tc.psum_pool(name='psc', bufs=6))

        def load(shape, dtype, src, name, eng=None):
            t = sing.tile(shape, dtype, name=name)
            (eng or nc.sync).dma_start(out=t[:], in_=src[:])
            return t

        id_sb = load([128, 128], F32, ident, 'id')
        # xwin split into 4 DMAs across two issue queues for transfer parallelism
        xwin_sb = sing.tile([128, 2, 38 * 64], FP16)
        for cg_ in range(2):
            for rh in range(2):
                eng = nc.sync if rh == 0 else nc.scalar
                r0, r1 = (0, 1216) if rh == 0 else (1216, 2432)
                eng.dma_start(out=_ap(xwin_sb[:], cg_ * 2432 + r0, [[2 * 2432, 128], [1, r1 - r0]]),
                              in_=_ap(xwin[:], cg_ * 128 * 2432 + r0, [[2432, 128], [1, r1 - r0]]))
        wc_sb = load([128, 2, 64], FP16, wc, 'wc')
        xT2_sb = sing.tile([128, 20 * 256], FP16)
        for rh in range(2):
            eng = nc.scalar if rh == 0 else nc.sync
            r0, r1 = (0, 2560) if rh == 0 else (2560, 5120)
            eng.dma_start(out=_ap(xT2_sb[:], r0, [[5120, 128], [1, r1 - r0]]),
                          in_=_ap(xT2[:], r0, [[5120, 128], [1, r1 - r0]]))
        wk_sb = load([128, 6 * 57], FP16, wk, 'wk', nc.scalar)
        par_sb = load([128, 40], F32, params, 'par')
        idx_sb = load([128, 100], I16, idxt, 'idx', nc.scalar)

        # PE warm-up while input DMAs land; dummy ACT pulls the table load early
        pw = psum.tile([128, 512], F32, name='warm', tag='front')
        dumt = work.tile([1, 4], F32, name='dumt')
        nc.scalar.activation(out=dumt[:], in_=id_sb[0:1, 0:4], func=AF.Copy, scale=1.0)
        for _ in range(NWARM):
            nc.tensor.matmul(pw[0:64, 0:64], id_sb[:, 0:64], id_sb[:, 0:64], start=True, stop=True)

        wvec = par_sb[:, P_WVEC:P_WVEC + 1]
        w63 = par_sb[:, P_W63:P_W63 + 1]
        bcomp = par_sb[0:64, P_BCOMP:P_BCOMP + 1]
        bker = _ap(par_sb[:], 32 * 40 + P_BCO, [[40, 25], [1, 1]])
        boff = par_sb[0:8, P_BCO:P_BCO + 1]
        hrow_bc = _ap(par_sb[:], P_HROW, [[40, 128], [1, 16], [0, 4]])
        y63_bc = _ap(par_sb[:], P_Y63, [[40, 128], [1, 16], [0, 4]])

        # hoisted shifted-variant buffers; edge partitions zeroed once (gpsimd)
        msm4 = work.tile([128, 20, 25, 4], FP16)
        msm4_p1 = work.tile([128, 20, 25, 4], FP16)   # [p] = msm4[p+1] within half
        msm4_m1 = work.tile([128, 20, 25, 4], FP16)   # [p] = msm4[p-1] within half
        for jh in range(2):
            nc.gpsimd.dma_start(
                out=_ap(msm4_p1[:], (jh * 64 + 63) * 2000, [[2000, 1], [1, 2000]]),
                in_=_ap(zed[:], 0, [[3600, 1], [1, 2000]]))
            nc.gpsimd.dma_start(
                out=_ap(msm4_m1[:], jh * 64 * 2000, [[2000, 1], [1, 2000]]),
                in_=_ap(zed[:], 0, [[3600, 1], [1, 2000]]))
        kernc = work.tile([128, 16 * 100], FP16)

        # ---- 1. compressor ----
        # cx_sb [128, 38, 66]: lower half = cx rows; upper half = cx shifted
        # down one h-row (slot h holds row h+1) so taps (dy=0, dy=1) pack
        # into one 128-deep contraction.
        cx_sb = work.tile([128, 38, 66], FP16)
        nc.vector.memset(_ap(cx_sb[:], 0, [[38 * 66, 128], [66, 38], [1, 1]]), 0.0)
        nc.vector.memset(_ap(cx_sb[:], 65, [[38 * 66, 128], [66, 38], [1, 1]]), 0.0)
        for grp in range(5):
            g0 = grp * 8
            rows = min(8, 38 - g0)
            nn = rows * 64
            pcs = psum.tile([64, 512], F32, name=f'cmp{grp}', tag='front')
            for cg in range(2):
                nc.tensor.matmul(pcs[:, :nn], wc_sb[:, cg, :],
                                 xwin_sb[:, cg, g0 * 64:g0 * 64 + nn],
                                 start=(cg == 0), stop=(cg == 1))
            nc.scalar.activation(
                out=_ap(cx_sb[:], g0 * 66 + 1, [[38 * 66, 64], [66, rows], [1, 64]]),
                in_=_ap(pcs[:], 0, [[512, 64], [64, rows], [1, 64]]),
                func=AF.Identity, bias=bcomp, scale=1.0)
            r0 = max(g0, 1)
            cnt = (g0 + rows - r0) * 66
            nc.gpsimd.dma_start(
                out=_ap(cx_sb[:], 64 * 2508 + (r0 - 1) * 66, [[2508, 64], [1, cnt]]),
                in_=_ap(cx_sb[:], r0 * 66, [[2508, 64], [1, cnt]]))

        # ---- 2. offset+mask convs: 6 MMs (3 tap-pairs + 3 singles) ----
        # expS [25, t20, jh2, 64]: slot (t, jh) = conv row h = t + 16*jh
        # (h in [16,20) stored twice). offS [8, h'16, jh2, 64]: y = h' + 16*jh.
        expS = work.tile([25, 20, 2, 64], F32)
        offS = work.tile([8, 16, 2, 64], F32)
        for grp in range(6):
            g0 = grp * 6
            nn = 6 * 64
            pcs = psum.tile([57, 384], F32, name=f'off{grp}', tag='front')
            for s in range(6):
                if s < 3:  # pair: lower tap (0,s), upper tap (1,s)
                    lhsT = _ap(wk_sb[:], s * 57, [[6 * 57, 128], [1, 57]])
                    rhs = _ap(cx_sb[:], g0 * 66 + s, [[38 * 66, 128], [66, 6], [1, 64]])
                else:      # single: tap (2, s-3), lower half only
                    lhsT = _ap(wk_sb[:], s * 57, [[6 * 57, 64], [1, 57]])
                    rhs = _ap(cx_sb[:], (g0 + 2) * 66 + (s - 3),
                              [[38 * 66, 64], [66, 6], [1, 64]])
                nc.tensor.matmul(pcs[:, :nn], lhsT, rhs,
                                 start=(s == 0), stop=(s == 5))
            for jh in range(2):
                h_lo = max(g0, 20 * jh - 4)      # jh0: t=h in [0,20); jh1: t=h-16
                h_hi = min(g0 + 6, 20 + 16 * jh)
                if h_lo < h_hi:
                    nc.scalar.activation(
                        out=_ap(expS[:], (h_lo - 16 * jh) * 128 + jh * 64,
                                [[2560, 25], [128, h_hi - h_lo], [1, 64]]),
                        in_=_ap(pcs[:], 32 * 384 + (h_lo - g0) * 64,
                                [[384, 25], [64, h_hi - h_lo], [1, 64]]),
                        func=AF.Exp, bias=bker, scale=1.0)
                y_lo = max(g0 - 2, 16 * jh)
                y_hi = min(g0 + 4, 16 + 16 * jh)
                if y_lo < y_hi:
                    nc.vector.tensor_scalar(
                        out=_ap(offS[:], (y_lo - 16 * jh) * 128 + jh * 64,
                                [[2048, 8], [128, y_hi - y_lo], [1, 64]]),
                        in0=_ap(pcs[:], (y_lo + 2 - g0) * 64,
                                [[384, 8], [64, y_hi - y_lo], [1, 64]]),
                        scalar1=boff, scalar2=None, op0=OP.add)

        # ---- 3. offset transposes -> deltT128 [128, 16 h', 8 ch] ----
        po = psum.tile([128, 512], F32, name='po', tag='front')
        for hp in range(16):
            nc.tensor.transpose(po[:, hp * 8:hp * 8 + 8],
                                _ap(offS[:], hp * 128, [[2048, 8], [1, 128]]),
                                id_sb[0:8, 0:8])
        deltT = work.tile([128, 16, 8], FP16)
        nc.scalar.activation(out=deltT[:], in_=_ap(po[:], 0, [[512, 128], [1, 128]]),
                             func=AF.Copy, scale=1.0)

        # ---- 4. W9 indicator chain on [128, 64] ----
        def dview(chbase):
            return _ap(deltT[:], chbase, [[128, 128], [8, 16], [1, 4]])

        def wt(nm):
            return work.tile([128, 64], FP16, name=nm)

        t1, t2 = wt('t1'), wt('t2')
        gxc, x0r, wxt, omwx, x1r = wt('gxc'), wt('x0r'), wt('wxt'), wt('omwx'), wt('x1r')
        gyc, y0r, wyt, omwy, y1r = wt('gyc'), wt('y0r'), wt('wyt'), wt('omwy'), wt('y1r')
        ia, ib = wt('ia'), wt('ib')
        cwx = work.tile([128, 3, 64], FP16)
        rwy = work.tile([128, 3, 64], FP16)
        W9b = work.tile([128, 9, 64], FP16)

        def r4(ap):
            return _ap(ap, 0, [[64, 128], [4, 16], [1, 4]])

        nc.vector.tensor_scalar(out=t1[:], in0=dview(0), scalar1=wvec, scalar2=None, op0=OP.add)
        nc.vector.tensor_scalar(out=t2[:], in0=t1[:], scalar1=0.0, scalar2=63.0, op0=OP.max, op1=OP.min)
        nc.vector.tensor_scalar(out=gxc[:], in0=t2[:], scalar1=wvec, scalar2=None, op0=OP.subtract)
        nc.vector.tensor_scalar(out=x0r[:], in0=gxc[:], scalar1=0.0, scalar2=-1.0, op0=OP.is_lt, op1=OP.mult)
        nc.vector.tensor_tensor(out=wxt[:], in0=gxc[:], in1=x0r[:], op=OP.subtract)
        nc.vector.tensor_scalar(out=omwx[:], in0=wxt[:], scalar1=-1.0, scalar2=1.0, op0=OP.mult, op1=OP.add)
        nc.vector.tensor_scalar(out=x1r[:], in0=x0r[:], scalar1=1.0, scalar2=w63, op0=OP.add, op1=OP.min)

        nc.vector.tensor_tensor(out=r4(t1[:]), in0=dview(4), in1=hrow_bc, op=OP.add)
        nc.vector.tensor_scalar(out=t2[:], in0=t1[:], scalar1=0.0, scalar2=63.0, op0=OP.max, op1=OP.min)
        nc.vector.tensor_tensor(out=r4(gyc[:]), in0=r4(t2[:]), in1=hrow_bc, op=OP.subtract)
        nc.vector.tensor_scalar(out=y0r[:], in0=gyc[:], scalar1=0.0, scalar2=-1.0, op0=OP.is_lt, op1=OP.mult)
        nc.vector.tensor_tensor(out=wyt[:], in0=gyc[:], in1=y0r[:], op=OP.subtract)
        nc.vector.tensor_scalar(out=omwy[:], in0=wyt[:], scalar1=-1.0, scalar2=1.0, op0=OP.mult, op1=OP.add)
        nc.vector.tensor_scalar(out=t1[:], in0=y0r[:], scalar1=1.0, scalar2=None, op0=OP.add)
        nc.vector.tensor_tensor(out=r4(y1r[:]), in0=r4(t1[:]), in1=y63_bc, op=OP.min)

        # batched indicators: all 3 tap offsets at once on [128, 3, 64]
        e3_bc = _ap(par_sb[:], P_E3, [[40, 128], [1, 3], [0, 64]])
        ia3 = work.tile([128, 3, 64], FP16, name='ia3')
        ib3 = work.tile([128, 3, 64], FP16, name='ib3')

        def bc3(t):
            return _ap(t[:], 0, [[64, 128], [0, 3], [1, 64]])

        for r0, r1, w0, w1, outt in ((x0r, x1r, omwx, wxt, cwx), (y0r, y1r, omwy, wyt, rwy)):
            nc.vector.tensor_tensor(out=ia3[:], in0=bc3(r0), in1=e3_bc, op=OP.is_equal)
            nc.vector.tensor_tensor(out=ib3[:], in0=bc3(r1), in1=e3_bc, op=OP.is_equal)
            nc.vector.tensor_tensor(out=ia3[:], in0=ia3[:], in1=bc3(w0), op=OP.mult)
            nc.vector.tensor_tensor(out=ib3[:], in0=ib3[:], in1=bc3(w1), op=OP.mult)
            nc.vector.tensor_tensor(out=outt[:], in0=ia3[:], in1=ib3[:], op=OP.add)
        for iy in range(3):
            for ix in range(3):
                nc.vector.tensor_tensor(
                    out=_ap(W9b[:], (iy * 3 + ix) * 64, [[9 * 64, 128], [1, 64]]),
                    in0=rwy[:, iy, :], in1=cwx[:, ix, :], op=OP.mult)

        # ---- 5. exp transposes -> expT128 [128, 20 t, 25 k]; softmax ----
        pt = psum.tile([128, 512], F32, name='pt', tag='front')
        for t in range(20):
            nc.tensor.transpose(pt[:, t * 25:t * 25 + 25],
                                _ap(expS[:], t * 128, [[2560, 25], [1, 128]]),
                                id_sb[0:25, 0:25])
        expT = work.tile([128, 20, 25], F32)
        nc.scalar.activation(out=expT[:], in_=_ap(pt[:], 0, [[512, 128], [1, 500]]),
                             func=AF.Copy, scale=1.0)
        sumT = work.tile([128, 20], F32)
        nc.vector.tensor_reduce(out=sumT[:], in_=expT[:], axis=mybir.AxisListType.X, op=OP.add)
        recT = work.tile([128, 20], F32)
        nc.vector.reciprocal(out=recT[:], in_=sumT[:])
        nc.vector.tensor_tensor(
            out=msm4[:],
            in0=_ap(expT[:], 0, [[500, 128], [25, 20], [1, 25], [0, 4]]),
            in1=_ap(recT[:], 0, [[20, 128], [1, 20], [0, 25], [0, 4]]), op=OP.mult)
        shift_engs = (nc.gpsimd, nc.sync, nc.scalar, nc.gpsimd)
        for jh in range(2):
            b0 = jh * 64 * 2000
            shift_engs[2 * jh].dma_start(
                out=_ap(msm4_p1[:], b0, [[2000, 63], [1, 2000]]),
                in_=_ap(msm4[:], b0 + 2000, [[2000, 63], [1, 2000]]))
            shift_engs[2 * jh + 1].dma_start(
                out=_ap(msm4_m1[:], b0 + 2000, [[2000, 63], [1, 2000]]),
                in_=_ap(msm4[:], b0, [[2000, 63], [1, 2000]]))

        # ---- 6-9. kernc assembly + banded + carafe, 4 blocks of 4 m ----
        # kernc k-dim is kj-major (host permutes mask channels), so each tap's
        # (ki, u) values are 20 contiguous elems and the partition-shifted
        # data_all gather is a handful of small direct DMAs per block.
        msm_by_ex = {-1: msm4_m1, 0: msm4, 1: msm4_p1}
        data_all = work.tile([128, 16, 100], FP16)
        nc.gpsimd.memset(data_all[:], 0.0)
        pbuf = [work.tile([128, 400], FP16, name=f'pb{t}') for t in range(9)]

        def emit_asm(m0, gm):
            # 9 independent products, then a pairwise reduction tree
            kv = _ap(kernc[:], m0 * 100, [[1600, 128], [100, gm], [4, 25], [1, 4]])

            def pv(t):
                return _ap(pbuf[t][:], 0, [[400, 128], [100, gm], [4, 25], [1, 4]])

            for t, (ey, ex) in enumerate((ey, ex) for ey in (-1, 0, 1) for ex in (-1, 0, 1)):
                mv = _ap(msm_by_ex[ex][:], (2 + ey + m0) * 100,
                         [[2000, 128], [100, gm], [4, 25], [1, 4]])
                wv = _ap(W9b[:], t * 64 + m0 * 4,
                         [[9 * 64, 128], [4, gm], [0, 25], [1, 4]])
                nc.vector.tensor_tensor(out=pv(t), in0=wv, in1=mv, op=OP.mult)
            for a, b in ((0, 1), (2, 3), (4, 5), (6, 7), (0, 2), (4, 6), (0, 4)):
                nc.vector.tensor_tensor(out=pv(a), in0=pv(a), in1=pv(b), op=OP.add)
            nc.vector.tensor_tensor(out=kv, in0=pv(0), in1=pv(8), op=OP.add)

        def emit_shift(m0, gm):
            # data_all[p, m, b*20:+20] = kernc[p + (b-2), m, (4-b)*20:+20]
            engs = {-2: nc.sync, -1: nc.scalar, 1: nc.sync, 2: nc.scalar}
            for b in (0, 1, 3, 4):
                d = b - 2
                eng = engs[d]
                for jh in range(2):
                    p_dst = jh * 64 + max(0, -d)
                    p_src = jh * 64 + max(0, d)
                    eng.dma_start(
                        out=_ap(data_all[:], p_dst * 1600 + m0 * 100 + b * 20,
                                [[1600, 64 - abs(d)], [100, gm], [1, 20]]),
                        in_=_ap(kernc[:], p_src * 1600 + m0 * 100 + (4 - b) * 20,
                                [[1600, 64 - abs(d)], [100, gm], [1, 20]]))
            nc.vector.tensor_copy(
                out=_ap(data_all[:], m0 * 100 + 2 * 20, [[1600, 128], [100, gm], [1, 20]]),
                in_=_ap(kernc[:], m0 * 100 + 2 * 20, [[1600, 128], [100, gm], [1, 20]]))

        rb_t = [None, None]

        def emit_m(m):
            banded = band.tile([128, 1280], FP16, name=f'band_{m}', tag='band')
            nc.gpsimd.local_scatter(out_ap=banded[:], data_ap=data_all[:, m, :],
                                    idxs_ap=idx_sb[:], channels=128, num_elems=1280,
                                    num_idxs=100)
            if m % 2 == 0:
                g = m // 2
                for ch in range(2):
                    rb_t[ch] = rbp.tile([128, 2 * 512], FP16, name=f'rb_{g}_{ch}', tag=f'rb{ch}')
            for jh in range(2):
                for ch in range(2):
                    pcs = psc.tile([128, 256], F32, name=f'pcs_{m}_{jh}_{ch}', tag='pcs')
                    for ki in range(5):
                        lhsT = _ap(xT2_sb[:], jh * 64 * 5120 + (m + ki) * 256 + ch * 128,
                                   [[5120, 64], [1, 128]])
                        rhs = _ap(banded[:], jh * 64 * 1280 + ki * 256, [[1280, 64], [1, 256]])
                        nc.tensor.matmul(pcs[:], lhsT, rhs, start=(ki == 0), stop=(ki == 4))
                    out_ap = _ap(rb_t[ch][:], (m % 2) * 512 + jh,
                                 [[2 * 512, 128], [128, 4], [8, 16], [2, 4]])
                    in_ap = _ap(pcs[:], 0, [[256, 128], [64, 4], [4, 16], [1, 4]])
                    if jh == 1 and m >= 8:
                        nc.vector.tensor_copy(out=out_ap, in_=in_ap)
                    else:
                        nc.scalar.activation(out=out_ap, in_=in_ap, func=AF.Copy, scale=1.0)
            if m % 2 == 1:
                for ch in range(2):
                    nc.sync.dma_start(
                        out=_ap(outp[:], ch * 128 * 8192 + 4 * (m - 1) * 128,
                                [[8192, 128], [128, 8], [1, 128]]),
                        in_=rb_t[ch][:])

        for m0, gm in ((0, 4), (4, 4), (8, 4), (12, 4)):
            emit_asm(m0, gm)
            emit_shift(m0, gm)
            for m in range(m0, m0 + gm):
                emit_m(m)
    nc.finalize()
    return nc


_PROGRAM = None
_SCAT = build_scatter_table()


def _get_program():
    global _PROGRAM
    if _PROGRAM is None:
        _PROGRAM = build_program()
    return _PROGRAM


def _prep_core_inputs(inputs, n, s):
    bf = np.float16
    x = np.asarray(inputs['x'][n], np.float32)
    h0 = 32 * s
    xw = np.zeros((C, 38, W), np.float32)
    for i, g in enumerate(range(h0 - 3, h0 + 35)):
        if 0 <= g < H:
            xw[:, i] = x[:, g]
    xwin = np.ascontiguousarray(xw.reshape(2, 128, 38 * 64)).astype(bf)
    xT2 = np.zeros((128, 20, C), np.float32)
    for jh in range(2):
        base = h0 + 16 * jh - 2
        for i in range(20):
            g = base + i
            if 0 <= g < H:
                xT2[64 * jh:64 * jh + 64, i] = x[:, g].T
    xT2 = np.ascontiguousarray(xT2.reshape(128, 20 * 256)).astype(bf)
    w_comp = np.asarray(inputs['w_comp'], np.float32)[:, :, 0, 0]
    wc = np.zeros((2, 128, 64), np.float32)
    for cg in range(2):
        wc[cg] = w_comp[:, cg * 128:(cg + 1) * 128].T
    wc = np.ascontiguousarray(wc.transpose(1, 0, 2).reshape(128, 2 * 64)).astype(bf)
    # mask channels permuted to kj-major so kernc taps are contiguous
    perm = [(c % 5) * 5 + c // 5 for c in range(25)]
    w_ker = np.asarray(inputs['w_ker'], np.float32)[perm]
    w_off = np.asarray(inputs['w_off'], np.float32)
    wkT = np.zeros((9, 64, 57), np.float32)
    for t in range(9):
        wkT[t, :, 0:8] = w_off[:, :, t // 3, t % 3].T
        wkT[t, :, 32:57] = w_ker[:, :, t // 3, t % 3].T
    wk = np.zeros((128, 6, 57), np.float32)
    for s, t in enumerate((0, 1, 2, 6, 7, 8)):
        wk[0:64, s] = wkT[t]
    for s, t in enumerate((3, 4, 5)):
        wk[64:128, s] = wkT[t]
    wk = np.ascontiguousarray(wk.reshape(128, 6 * 57)).astype(bf)

    par = np.zeros((128, 40), np.float32)
    p = np.arange(128)
    wv = (p % 64).astype(np.float32)
    jh = (p // 64).astype(np.float32)
    par[:, P_WVEC] = wv
    par[:, P_W63] = 63.0 - wv
    hh = h0 + 16.0 * jh[:, None] + np.arange(16, dtype=np.float32)[None, :]
    par[:, P_HROW:P_HROW + 16] = hh
    par[:, P_Y63:P_Y63 + 16] = 63.0 - hh
    par[0:64, P_BCOMP] = np.asarray(inputs['b_comp'], np.float32)
    bcov = np.zeros(128, np.float32)
    bcov[0:8] = np.asarray(inputs['b_off'], np.float32)
    bcov[32:57] = np.asarray(inputs['b_ker'], np.float32)[perm]
    par[:, P_BCO] = bcov
    par[:, P_E3:P_E3 + 3] = np.array([-1.0, 0.0, 1.0], np.float32)

    return {
        'xwin': xwin, 'xT2': xT2, 'wc': wc, 'wk': wk, 'params': par,
        'ident': np.eye(128, dtype=np.float32),
        'idxt': _SCAT,
        'zed': np.zeros((2, 3600), np.float16),
    }


def kernel(**inputs):
    nc = _get_program()
    core_ids = list(range(8))
    in_maps = [_prep_core_inputs(inputs, cid // 2, cid % 2) for cid in core_ids]
    res = run_bass_kernel_spmd(nc, in_maps, core_ids)
    out = np.zeros((N, C, 128, 128), np.float32)
    for cid in core_ids:
        n, s = cid // 2, cid % 2
        op = np.asarray(res.results[cid]['outp']).astype(np.float32).reshape(256, 64, 128)
        out[n, :, s::2] = op
    return out


if __name__ == '__main__':
    d = np.load('/root/problem/ref_io.npz')
    inp = {k: d[k] for k in ('x', 'w_comp', 'b_comp', 'w_ker', 'b_ker', 'w_off', 'b_off')}
    out = kernel(**inp)
    ref = d['out']
    err = np.abs(out - ref).max()
    print('max abs err:', err, 'rel:', err / np.abs(ref).max())


# revision 8
# speedup vs baseline: 1.0644x; 1.0439x over previous
"""Trainium2 Bass kernel for nn_DLUPack (CARAFE-style dynamic upsampling), v2.

Sharding: 8 cores = (batch n in [0,4)) x (output-row-parity s in {0,1});
core (n, s) computes low-res rows hh in [32s, 32s+32) -> all parity-s output rows.

v2 layout: back phase jh-packed on 128 partitions, p = 64*jh + w.
  ref[n, c, 2y+i, 2x+j]: for core (n,s), y = h0 + 16*jh + m (h0=32s),
  out DRAM row r' = 4m + w//16, dcol = 8*(w%16) + 2u + jh, host: out[n,:,s::2].

Pipeline per core:
  1. compressor 1x1 conv (PE) -> cx [64, 38, 66] fp16
  2. offset+mask 3x3 convs (9 accumulated MMs x 6 groups) -> psum [57, 384]
  3. 16 po transposes -> deltT128 [128, 16, 8]; W9 indicator chain (DVE)
  4. 20 exp transposes -> expT128 [128, 20, 25] f32; softmax; msm4 [128,20,25,4] fp16
  5. +-1 w-shift variants of msm4 via SBUF-SBUF DMA (within 64-halves)
  6. kernc [128, 16m, 25k, 4u] assembly: 17 TT ops x 2 blocks (DVE, fp16 2x)
  7. kbf partition-shift variants (4 DMAs/blk); prep -> data_all [128, 16, 100]
  8. per m: local_scatter [128, 1280] (GPSIMD); per (jh, ch): 5 accumulated MMs
     lhsT=xT2[64jh.., 128c] rhs=banded[64jh.., ki*256..] -> psum [128c, 256px]
  9. ACT evac (fp16, jh-interleaved cols) -> rb group tile; 1 out DMA per (4m, ch)
"""
import sys
import numpy as np

sys.path.insert(0, '/opt/trn_rl_repo')

import ml_dtypes  # noqa: E402,F401
from contextlib import ExitStack  # noqa: E402

import concourse.bass as bass  # noqa: E402
import concourse.tile as tile  # noqa: E402
from concourse import mybir, bacc  # noqa: E402
from concourse.bass_utils import run_bass_kernel_spmd  # noqa: E402

F32 = mybir.dt.float32
FP16 = mybir.dt.float16
I16 = mybir.dt.int16
AF = mybir.ActivationFunctionType
OP = mybir.AluOpType

N, C, H, W = 4, 256, 64, 64
NWARM = 24


def _ap(base, off_elems, dims):
    return bass.AP(tensor=base.tensor, offset=base.offset + off_elems, ap=[list(d) for d in dims])


def build_scatter_table():
    # banded[p=64jh+pp, ki*256 + 4*w + u] = kernc[64jh + w, m, ki*5+(4-b), u],
    # w = pp + b - 2; data_all[p, (b*5+ki)*4+u] laid out by prep.
    idx = -np.ones((128, 100), np.int16)
    for p in range(128):
        pp = p % 64
        for b in range(5):
            w = pp + b - 2
            if not (0 <= w < 64):
                continue
            for ki in range(5):
                for u in range(4):
                    idx[p, (b * 5 + ki) * 4 + u] = ki * 256 + 4 * w + u
    return idx


# params [128, 40] f32 column map
P_WVEC, P_W63, P_HROW, P_Y63, P_BCOMP, P_BCO, P_E3 = 0, 1, 2, 18, 34, 35, 36


def build_program():
    nc = bacc.Bacc(None, target_bir_lowering=False, debug=True)

    xwin = nc.declare_dram_parameter('xwin', [2, 128, 38 * 64], FP16, isOutput=False)
    xT2 = nc.declare_dram_parameter('xT2', [128, 20 * 256], FP16, isOutput=False)
    wc = nc.declare_dram_parameter('wc', [128, 2 * 64], FP16, isOutput=False)
    wk = nc.declare_dram_parameter('wk', [128, 6 * 57], FP16, isOutput=False)
    params = nc.declare_dram_parameter('params', [128, 40], F32, isOutput=False)
    ident = nc.declare_dram_parameter('ident', [128, 128], F32, isOutput=False)
    idxt = nc.declare_dram_parameter('idxt', [128, 100], I16, isOutput=False)
    zed = nc.declare_dram_parameter('zed', [2, 3600], FP16, isOutput=False)
    outp = nc.declare_dram_parameter('outp', [256, 64 * 128], FP16, isOutput=True)

    with tile.TileContext(nc) as tc, ExitStack() as ctx:
        sing = ctx.enter_context(tc.tile_pool(name='sing', bufs=1))
        work = ctx.enter_context(tc.tile_pool(name='work', bufs=1))
        band = ctx.enter_context(tc.tile_pool(name='band', bufs=4))
        rbp = ctx.enter_context(tc.tile_pool(name='rbp', bufs=2))
        psum = ctx.enter_context(tc.psum_pool(name='ps', bufs=2))
        psc = ctx.enter_context(tc.psum_pool(name='psc', bufs=6))

        def load(shape, dtype, src, name, eng=None):
            t = sing.tile(shape, dtype, name=name)
            (eng or nc.sync).dma_start(out=t[:], in_=src[:])
            return t

        id_sb = load([128, 128], F32, ident, 'id')
        # xwin split into 4 DMAs across two issue queues for transfer parallelism
        xwin_sb = sing.tile([128, 2, 38 * 64], FP16)
        for cg_ in range(2):
            for rh in range(2):
                eng = nc.sync if rh == 0 else nc.scalar
                r0, r1 = (0, 1216) if rh == 0 else (1216, 2432)
                eng.dma_start(out=_ap(xwin_sb[:], cg_ * 2432 + r0, [[2 * 2432, 128], [1, r1 - r0]]),
                              in_=_ap(xwin[:], cg_ * 128 * 2432 + r0, [[2432, 128], [1, r1 - r0]]))
        wc_sb = load([128, 2, 64], FP16, wc, 'wc')
        wk_sb = load([128, 6 * 57], FP16, wk, 'wk', nc.scalar)
        par_sb = load([128, 40], F32, params, 'par')
        idx_sb = load([128, 100], I16, idxt, 'idx', nc.scalar)
        # xT2 loaded LAST: not needed until the carafe MMs, and its transfer
        # landing during the conv phase steals SBUF write bandwidth from PE
        xT2_sb = sing.tile([128, 20 * 256], FP16)
        for rh in range(2):
            eng = nc.scalar if rh == 0 else nc.sync
            r0, r1 = (0, 2560) if rh == 0 else (2560, 5120)
            eng.dma_start(out=_ap(xT2_sb[:], r0, [[5120, 128], [1, r1 - r0]]),
                          in_=_ap(xT2[:], r0, [[5120, 128], [1, r1 - r0]]))

        # PE warm-up while input DMAs land; dummy ACT pulls the table load early
        pw = psum.tile([128, 512], F32, name='warm', tag='front')
        dumt = work.tile([1, 4], F32, name='dumt')
        nc.scalar.activation(out=dumt[:], in_=id_sb[0:1, 0:4], func=AF.Copy, scale=1.0)
        for _ in range(NWARM):
            nc.tensor.matmul(pw[0:64, 0:64], id_sb[:, 0:64], id_sb[:, 0:64], start=True, stop=True)

        wvec = par_sb[:, P_WVEC:P_WVEC + 1]
        w63 = par_sb[:, P_W63:P_W63 + 1]
        bcomp = par_sb[0:64, P_BCOMP:P_BCOMP + 1]
        bker = _ap(par_sb[:], 32 * 40 + P_BCO, [[40, 25], [1, 1]])
        boff = par_sb[0:8, P_BCO:P_BCO + 1]
        hrow_bc = _ap(par_sb[:], P_HROW, [[40, 128], [1, 16], [0, 4]])
        y63_bc = _ap(par_sb[:], P_Y63, [[40, 128], [1, 16], [0, 4]])

        # hoisted shifted-variant buffers; edge partitions zeroed once (gpsimd)
        msm4 = work.tile([128, 20, 25, 4], FP16)
        msm4_p1 = work.tile([128, 20, 25, 4], FP16)   # [p] = msm4[p+1] within half
        msm4_m1 = work.tile([128, 20, 25, 4], FP16)   # [p] = msm4[p-1] within half
        for jh in range(2):
            nc.gpsimd.dma_start(
                out=_ap(msm4_p1[:], (jh * 64 + 63) * 2000, [[2000, 1], [1, 2000]]),
                in_=_ap(zed[:], 0, [[3600, 1], [1, 2000]]))
            nc.gpsimd.dma_start(
                out=_ap(msm4_m1[:], jh * 64 * 2000, [[2000, 1], [1, 2000]]),
                in_=_ap(zed[:], 0, [[3600, 1], [1, 2000]]))
        kernc = work.tile([128, 16 * 100], FP16)

        # ---- 1. compressor ----
        # cx_sb [128, 38, 66]: lower half = cx rows; upper half = cx shifted
        # down one h-row (slot h holds row h+1) so taps (dy=0, dy=1) pack
        # into one 128-deep contraction.
        cx_sb = work.tile([128, 38, 66], FP16)
        nc.vector.memset(_ap(cx_sb[:], 0, [[38 * 66, 128], [66, 38], [1, 1]]), 0.0)
        nc.vector.memset(_ap(cx_sb[:], 65, [[38 * 66, 128], [66, 38], [1, 1]]), 0.0)
        for grp in range(5):
            g0 = grp * 8
            rows = min(8, 38 - g0)
            nn = rows * 64
            pcs = psum.tile([64, 512], F32, name=f'cmp{grp}', tag='front')
            for cg in range(2):
                nc.tensor.matmul(pcs[:, :nn], wc_sb[:, cg, :],
                                 xwin_sb[:, cg, g0 * 64:g0 * 64 + nn],
                                 start=(cg == 0), stop=(cg == 1))
            nc.scalar.activation(
                out=_ap(cx_sb[:], g0 * 66 + 1, [[38 * 66, 64], [66, rows], [1, 64]]),
                in_=_ap(pcs[:], 0, [[512, 64], [64, rows], [1, 64]]),
                func=AF.Identity, bias=bcomp, scale=1.0)
            r0 = max(g0, 1)
            cnt = (g0 + rows - r0) * 66
            nc.gpsimd.dma_start(
                out=_ap(cx_sb[:], 64 * 2508 + (r0 - 1) * 66, [[2508, 64], [1, cnt]]),
                in_=_ap(cx_sb[:], r0 * 66, [[2508, 64], [1, cnt]]))

        # ---- 2. offset+mask convs: 6 MMs (3 tap-pairs + 3 singles) ----
        # expS [25, t20, jh2, 64]: slot (t, jh) = conv row h = t + 16*jh
        # (h in [16,20) stored twice). offS [8, h'16, jh2, 64]: y = h' + 16*jh.
        expS = work.tile([25, 20, 2, 64], F32)
        offS = work.tile([8, 16, 2, 64], F32)
        for grp in range(6):
            g0 = grp * 6
            nn = 6 * 64
            pcs = psum.tile([57, 384], F32, name=f'off{grp}', tag='front')
            for s in range(6):
                if s < 3:  # pair: lower tap (0,s), upper tap (1,s)
                    lhsT = _ap(wk_sb[:], s * 57, [[6 * 57, 128], [1, 57]])
                    rhs = _ap(cx_sb[:], g0 * 66 + s, [[38 * 66, 128], [66, 6], [1, 64]])
                else:      # single: tap (2, s-3), lower half only
                    lhsT = _ap(wk_sb[:], s * 57, [[6 * 57, 64], [1, 57]])
                    rhs = _ap(cx_sb[:], (g0 + 2) * 66 + (s - 3),
                              [[38 * 66, 64], [66, 6], [1, 64]])
                nc.tensor.matmul(pcs[:, :nn], lhsT, rhs,
                                 start=(s == 0), stop=(s == 5))
            for jh in range(2):
                h_lo = max(g0, 20 * jh - 4)      # jh0: t=h in [0,20); jh1: t=h-16
                h_hi = min(g0 + 6, 20 + 16 * jh)
                if h_lo < h_hi:
                    nc.scalar.activation(
                        out=_ap(expS[:], (h_lo - 16 * jh) * 128 + jh * 64,
                                [[2560, 25], [128, h_hi - h_lo], [1, 64]]),
                        in_=_ap(pcs[:], 32 * 384 + (h_lo - g0) * 64,
                                [[384, 25], [64, h_hi - h_lo], [1, 64]]),
                        func=AF.Exp, bias=bker, scale=1.0)
                y_lo = max(g0 - 2, 16 * jh)
                y_hi = min(g0 + 4, 16 + 16 * jh)
                if y_lo < y_hi:
                    nc.vector.tensor_scalar(
                        out=_ap(offS[:], (y_lo - 16 * jh) * 128 + jh * 64,
                                [[2048, 8], [128, y_hi - y_lo], [1, 64]]),
                        in0=_ap(pcs[:], (y_lo + 2 - g0) * 64,
                                [[384, 8], [64, y_hi - y_lo], [1, 64]]),
                        scalar1=boff, scalar2=None, op0=OP.add)

        # ---- 3. offset transposes -> deltT128 [128, 16 h', 8 ch] ----
        po = psum.tile([128, 512], F32, name='po', tag='front')
        for hp in range(16):
            nc.tensor.transpose(po[:, hp * 8:hp * 8 + 8],
                                _ap(offS[:], hp * 128, [[2048, 8], [1, 128]]),
                                id_sb[0:8, 0:8])
        deltT = work.tile([128, 16, 8], FP16)
        nc.scalar.activation(out=deltT[:], in_=_ap(po[:], 0, [[512, 128], [1, 128]]),
                             func=AF.Copy, scale=1.0)

        # ---- 4. W9 indicator chain on [128, 64] ----
        def dview(chbase):
            return _ap(deltT[:], chbase, [[128, 128], [8, 16], [1, 4]])

        def wt(nm):
            return work.tile([128, 64], FP16, name=nm)

        t1, t2 = wt('t1'), wt('t2')
        gxc, x0r, wxt, omwx, x1r = wt('gxc'), wt('x0r'), wt('wxt'), wt('omwx'), wt('x1r')
        gyc, y0r, wyt, omwy, y1r = wt('gyc'), wt('y0r'), wt('wyt'), wt('omwy'), wt('y1r')
        ia, ib = wt('ia'), wt('ib')
        cwx = work.tile([128, 3, 64], FP16)
        rwy = work.tile([128, 3, 64], FP16)
        W9b = work.tile([128, 9, 64], FP16)

        def r4(ap):
            return _ap(ap, 0, [[64, 128], [4, 16], [1, 4]])

        nc.vector.tensor_scalar(out=t1[:], in0=dview(0), scalar1=wvec, scalar2=None, op0=OP.add)
        nc.vector.tensor_scalar(out=t2[:], in0=t1[:], scalar1=0.0, scalar2=63.0, op0=OP.max, op1=OP.min)
        nc.vector.tensor_scalar(out=gxc[:], in0=t2[:], scalar1=wvec, scalar2=None, op0=OP.subtract)
        nc.vector.tensor_scalar(out=x0r[:], in0=gxc[:], scalar1=0.0, scalar2=-1.0, op0=OP.is_lt, op1=OP.mult)
        nc.vector.tensor_tensor(out=wxt[:], in0=gxc[:], in1=x0r[:], op=OP.subtract)
        nc.vector.tensor_scalar(out=omwx[:], in0=wxt[:], scalar1=-1.0, scalar2=1.0, op0=OP.mult, op1=OP.add)
        nc.vector.tensor_scalar(out=x1r[:], in0=x0r[:], scalar1=1.0, scalar2=w63, op0=OP.add, op1=OP.min)

        nc.vector.tensor_tensor(out=r4(t1[:]), in0=dview(4), in1=hrow_bc, op=OP.add)
        nc.vector.tensor_scalar(out=t2[:], in0=t1[:], scalar1=0.0, scalar2=63.0, op0=OP.max, op1=OP.min)
        nc.vector.tensor_tensor(out=r4(gyc[:]), in0=r4(t2[:]), in1=hrow_bc, op=OP.subtract)
        nc.vector.tensor_scalar(out=y0r[:], in0=gyc[:], scalar1=0.0, scalar2=-1.0, op0=OP.is_lt, op1=OP.mult)
        nc.vector.tensor_tensor(out=wyt[:], in0=gyc[:], in1=y0r[:], op=OP.subtract)
        nc.vector.tensor_scalar(out=omwy[:], in0=wyt[:], scalar1=-1.0, scalar2=1.0, op0=OP.mult, op1=OP.add)
        nc.vector.tensor_scalar(out=t1[:], in0=y0r[:], scalar1=1.0, scalar2=None, op0=OP.add)
        nc.vector.tensor_tensor(out=r4(y1r[:]), in0=r4(t1[:]), in1=y63_bc, op=OP.min)

        # batched indicators: all 3 tap offsets at once on [128, 3, 64]
        e3_bc = _ap(par_sb[:], P_E3, [[40, 128], [1, 3], [0, 64]])
        ia3 = work.tile([128, 3, 64], FP16, name='ia3')
        ib3 = work.tile([128, 3, 64], FP16, name='ib3')

        def bc3(t):
            return _ap(t[:], 0, [[64, 128], [0, 3], [1, 64]])

        for r0, r1, w0, w1, outt in ((x0r, x1r, omwx, wxt, cwx), (y0r, y1r, omwy, wyt, rwy)):
            nc.vector.tensor_tensor(out=ia3[:], in0=bc3(r0), in1=e3_bc, op=OP.is_equal)
            nc.vector.tensor_tensor(out=ib3[:], in0=bc3(r1), in1=e3_bc, op=OP.is_equal)
            nc.vector.tensor_tensor(out=ia3[:], in0=ia3[:], in1=bc3(w0), op=OP.mult)
            nc.vector.tensor_tensor(out=ib3[:], in0=ib3[:], in1=bc3(w1), op=OP.mult)
            nc.vector.tensor_tensor(out=outt[:], in0=ia3[:], in1=ib3[:], op=OP.add)
        for iy in range(3):
            for ix in range(3):
                nc.vector.tensor_tensor(
                    out=_ap(W9b[:], (iy * 3 + ix) * 64, [[9 * 64, 128], [1, 64]]),
                    in0=rwy[:, iy, :], in1=cwx[:, ix, :], op=OP.mult)

        # ---- 5. exp transposes -> expT128 [128, 20 t, 25 k]; softmax ----
        pt = psum.tile([128, 512], F32, name='pt', tag='front')
        for t in range(20):
            nc.tensor.transpose(pt[:, t * 25:t * 25 + 25],
                                _ap(expS[:], t * 128, [[2560, 25], [1, 128]]),
                                id_sb[0:25, 0:25])
        expT = work.tile([128, 20, 25], F32)
        nc.scalar.activation(out=expT[:], in_=_ap(pt[:], 0, [[512, 128], [1, 500]]),
                             func=AF.Copy, scale=1.0)
        sumT = work.tile([128, 20], F32)
        nc.vector.tensor_reduce(out=sumT[:], in_=expT[:], axis=mybir.AxisListType.X, op=OP.add)
        recT = work.tile([128, 20], F32)
        nc.vector.reciprocal(out=recT[:], in_=sumT[:])
        nc.vector.tensor_tensor(
            out=msm4[:],
            in0=_ap(expT[:], 0, [[500, 128], [25, 20], [1, 25], [0, 4]]),
            in1=_ap(recT[:], 0, [[20, 128], [1, 20], [0, 25], [0, 4]]), op=OP.mult)
        shift_engs = (nc.gpsimd, nc.sync, nc.scalar, nc.gpsimd)
        for jh in range(2):
            b0 = jh * 64 * 2000
            shift_engs[2 * jh].dma_start(
                out=_ap(msm4_p1[:], b0, [[2000, 63], [1, 2000]]),
                in_=_ap(msm4[:], b0 + 2000, [[2000, 63], [1, 2000]]))
            shift_engs[2 * jh + 1].dma_start(
                out=_ap(msm4_m1[:], b0 + 2000, [[2000, 63], [1, 2000]]),
                in_=_ap(msm4[:], b0, [[2000, 63], [1, 2000]]))

        # ---- 6-9. kernc assembly + banded + carafe, 4 blocks of 4 m ----
        # kernc k-dim is kj-major (host permutes mask channels), so each tap's
        # (ki, u) values are 20 contiguous elems and the partition-shifted
        # data_all gather is a handful of small direct DMAs per block.
        msm_by_ex = {-1: msm4_m1, 0: msm4, 1: msm4_p1}
        data_all = work.tile([128, 16, 100], FP16)
        nc.gpsimd.memset(data_all[:], 0.0)
        pbuf = [work.tile([128, 400], FP16, name=f'pb{t}') for t in range(9)]

        def emit_asm(m0, gm):
            # 9 independent products, then a pairwise reduction tree
            kv = _ap(kernc[:], m0 * 100, [[1600, 128], [100, gm], [4, 25], [1, 4]])

            def pv(t):
                return _ap(pbuf[t][:], 0, [[400, 128], [100, gm], [4, 25], [1, 4]])

            for t, (ey, ex) in enumerate((ey, ex) for ey in (-1, 0, 1) for ex in (-1, 0, 1)):
                mv = _ap(msm_by_ex[ex][:], (2 + ey + m0) * 100,
                         [[2000, 128], [100, gm], [4, 25], [1, 4]])
                wv = _ap(W9b[:], t * 64 + m0 * 4,
                         [[9 * 64, 128], [4, gm], [0, 25], [1, 4]])
                nc.vector.tensor_tensor(out=pv(t), in0=wv, in1=mv, op=OP.mult)
            for a, b in ((0, 1), (2, 3), (4, 5), (6, 7), (0, 2), (4, 6), (0, 4)):
                nc.vector.tensor_tensor(out=pv(a), in0=pv(a), in1=pv(b), op=OP.add)
            nc.vector.tensor_tensor(out=kv, in0=pv(0), in1=pv(8), op=OP.add)

        def emit_shift(m0, gm):
            # data_all[p, m, b*20:+20] = kernc[p + (b-2), m, (4-b)*20:+20]
            # early blocks issue off the sync queue (gpsimd idle pre-scatter)
            if m0 < 8:
                engs = {-2: nc.gpsimd, -1: nc.scalar, 1: nc.gpsimd, 2: nc.scalar}
            else:
                engs = {-2: nc.sync, -1: nc.scalar, 1: nc.sync, 2: nc.scalar}
            for b in (0, 1, 3, 4):
                d = b - 2
                eng = engs[d]
                for jh in range(2):
                    p_dst = jh * 64 + max(0, -d)
                    p_src = jh * 64 + max(0, d)
                    eng.dma_start(
                        out=_ap(data_all[:], p_dst * 1600 + m0 * 100 + b * 20,
                                [[1600, 64 - abs(d)], [100, gm], [1, 20]]),
                        in_=_ap(kernc[:], p_src * 1600 + m0 * 100 + (4 - b) * 20,
                                [[1600, 64 - abs(d)], [100, gm], [1, 20]]))
            nc.vector.tensor_copy(
                out=_ap(data_all[:], m0 * 100 + 2 * 20, [[1600, 128], [100, gm], [1, 20]]),
                in_=_ap(kernc[:], m0 * 100 + 2 * 20, [[1600, 128], [100, gm], [1, 20]]))

        rb_t = [None, None]

        def emit_m(m):
            banded = band.tile([128, 1280], FP16, name=f'band_{m}', tag='band')
            nc.gpsimd.local_scatter(out_ap=banded[:], data_ap=data_all[:, m, :],
                                    idxs_ap=idx_sb[:], channels=128, num_elems=1280,
                                    num_idxs=100)
            if m % 4 == 0:
                g = m // 4
                for ch in range(2):
                    rb_t[ch] = rbp.tile([128, 4 * 512], FP16, name=f'rb_{g}_{ch}', tag=f'rb{ch}')
            for jh in range(2):
                for ch in range(2):
                    pcs = psc.tile([128, 256], F32, name=f'pcs_{m}_{jh}_{ch}', tag='pcs')
                    for ki in range(5):
                        lhsT = _ap(xT2_sb[:], jh * 64 * 5120 + (m + ki) * 256 + ch * 128,
                                   [[5120, 64], [1, 128]])
                        rhs = _ap(banded[:], jh * 64 * 1280 + ki * 256, [[1280, 64], [1, 256]])
                        nc.tensor.matmul(pcs[:], lhsT, rhs, start=(ki == 0), stop=(ki == 4))
                    out_ap = _ap(rb_t[ch][:], (m % 4) * 512 + jh,
                                 [[4 * 512, 128], [128, 4], [8, 16], [2, 4]])
                    in_ap = _ap(pcs[:], 0, [[256, 128], [64, 4], [4, 16], [1, 4]])
                    if jh == 1 and m >= 8:
                        nc.vector.tensor_copy(out=out_ap, in_=in_ap)
                    else:
                        nc.scalar.activation(out=out_ap, in_=in_ap, func=AF.Copy, scale=1.0)
            if m % 4 == 3:
                for ch in range(2):
                    nc.sync.dma_start(
                        out=_ap(outp[:], ch * 128 * 8192 + 4 * (m - 3) * 128,
                                [[8192, 128], [128, 16], [1, 128]]),
                        in_=rb_t[ch][:])

        for m0, gm in ((0, 4), (4, 4), (8, 4), (12, 4)):
            emit_asm(m0, gm)
            emit_shift(m0, gm)
            for m in range(m0, m0 + gm):
                emit_m(m)
    nc.finalize()
    return nc


_PROGRAM = None
_SCAT = build_scatter_table()


def _get_program():
    global _PROGRAM
    if _PROGRAM is None:
        _PROGRAM = build_program()
    return _PROGRAM


def _prep_core_inputs(inputs, n, s):
    bf = np.float16
    x = np.asarray(inputs['x'][n], np.float32)
    h0 = 32 * s
    xw = np.zeros((C, 38, W), np.float32)
    for i, g in enumerate(range(h0 - 3, h0 + 35)):
        if 0 <= g < H:
            xw[:, i] = x[:, g]
    xwin = np.ascontiguousarray(xw.reshape(2, 128, 38 * 64)).astype(bf)
    xT2 = np.zeros((128, 20, C), np.float32)
    for jh in range(2):
        base = h0 + 16 * jh - 2
        for i in range(20):
            g = base + i
            if 0 <= g < H:
                xT2[64 * jh:64 * jh + 64, i] = x[:, g].T
    xT2 = np.ascontiguousarray(xT2.reshape(128, 20 * 256)).astype(bf)
    w_comp = np.asarray(inputs['w_comp'], np.float32)[:, :, 0, 0]
    wc = np.zeros((2, 128, 64), np.float32)
    for cg in range(2):
        wc[cg] = w_comp[:, cg * 128:(cg + 1) * 128].T
    wc = np.ascontiguousarray(wc.transpose(1, 0, 2).reshape(128, 2 * 64)).astype(bf)
    # mask channels permuted to kj-major so kernc taps are contiguous
    perm = [(c % 5) * 5 + c // 5 for c in range(25)]
    w_ker = np.asarray(inputs['w_ker'], np.float32)[perm]
    w_off = np.asarray(inputs['w_off'], np.float32)
    wkT = np.zeros((9, 64, 57), np.float32)
    for t in range(9):
        wkT[t, :, 0:8] = w_off[:, :, t // 3, t % 3].T
        wkT[t, :, 32:57] = w_ker[:, :, t // 3, t % 3].T
    wk = np.zeros((128, 6, 57), np.float32)
    for s, t in enumerate((0, 1, 2, 6, 7, 8)):
        wk[0:64, s] = wkT[t]
    for s, t in enumerate((3, 4, 5)):
        wk[64:128, s] = wkT[t]
    wk = np.ascontiguousarray(wk.reshape(128, 6 * 57)).astype(bf)

    par = np.zeros((128, 40), np.float32)
    p = np.arange(128)
    wv = (p % 64).astype(np.float32)
    jh = (p // 64).astype(np.float32)
    par[:, P_WVEC] = wv
    par[:, P_W63] = 63.0 - wv
    hh = h0 + 16.0 * jh[:, None] + np.arange(16, dtype=np.float32)[None, :]
    par[:, P_HROW:P_HROW + 16] = hh
    par[:, P_Y63:P_Y63 + 16] = 63.0 - hh
    par[0:64, P_BCOMP] = np.asarray(inputs['b_comp'], np.float32)
    bcov = np.zeros(128, np.float32)
    bcov[0:8] = np.asarray(inputs['b_off'], np.float32)
    bcov[32:57] = np.asarray(inputs['b_ker'], np.float32)[perm]
    par[:, P_BCO] = bcov
    par[:, P_E3:P_E3 + 3] = np.array([-1.0, 0.0, 1.0], np.float32)

    return {
        'xwin': xwin, 'xT2': xT2, 'wc': wc, 'wk': wk, 'params': par,
        'ident': np.eye(128, dtype=np.float32),
        'idxt': _SCAT,
        'zed': np.zeros((2, 3600), np.float16),
    }


def kernel(**inputs):
    nc = _get_program()
    core_ids = list(range(8))
    in_maps = [_prep_core_inputs(inputs, cid // 2, cid % 2) for cid in core_ids]
    res = run_bass_kernel_spmd(nc, in_maps, core_ids)
    out = np.zeros((N, C, 128, 128), np.float32)
    for cid in core_ids:
        n, s = cid // 2, cid % 2
        op = np.asarray(res.results[cid]['outp']).astype(np.float32).reshape(256, 64, 128)
        out[n, :, s::2] = op
    return out


if __name__ == '__main__':
    d = np.load('/root/problem/ref_io.npz')
    inp = {k: d[k] for k in ('x', 'w_comp', 'b_comp', 'w_ker', 'b_ker', 'w_off', 'b_off')}
    out = kernel(**inp)
    ref = d['out']
    err = np.abs(out - ref).max()
    print('max abs err:', err, 'rel:', err / np.abs(ref).max())
